# revision 1
# baseline (speedup 1.0000x reference)
"""MixLlamaMLP Trainium2 kernel.

y = (silu(x @ w_gate) * (x @ w_up)) @ w_down

Strategy: data-parallel over tokens across 8 NeuronCores (1024 tokens/core);
each core runs the full MLP on its token slice in bf16 (fp32 accumulate).
No collectives.

Host-side prep (not counted in HW time): weights cast to bf16 and packed so
every device DMA is fully contiguous per partition; x pre-transposed per core.

Device schedule per core:
  Stage A: h = silu(x@wg) * (x@wu), streaming packed wg/wu once.  The first
    512-token half of h is written straight into an SBUF slab (no DMA); the
    second half goes to DRAM.
  Stage B: y = h @ wd in two 512-token passes.  Pass 0 reads h from the
    resident slab (zero h DMA); pass 1 reloads the slab from DRAM while
    pass 0 drains.  w_down streams once per pass.  PSUM is ping-ponged in
    4-bank halves so block boundaries never stall the PE.

Self-contained: hardcodes shapes B=4, S=2048, H=4096, I=11008.
"""
import os

import numpy as np
import ml_dtypes

import concourse.bass as bass
import concourse.mybir as mybir
import concourse.tile as tile
from bass_rust import ScopedClock as _ScopedClock
from concourse.bass_utils import run_bass_kernel_spmd

# ---------------------------------------------------------------------------
# Patch: this walrus build only accepts ONE sync-wait command per CTRL (Drain)
# instruction; Tile's kernel-tail drain carries one wait per logical proc.
# Split the waits across a chain of drain instructions (drain is idempotent).
_MAX_DRAIN_WAITS = 1


def _split_drain_and_barrier(self, tick_clock, wait_clock):
    nc = self.nc
    drain_inst = nc.sync.drain()
    wait_clock.add_sem_waits(
        drain_inst.ins, _ScopedClock({None: tick_clock.global_clock})
    )
    si = drain_inst.ins.sync_info
    waits = list(si.on_wait) if si is not None and si.on_wait else []
    if len(waits) > _MAX_DRAIN_WAITS:
        si.on_wait = waits[:_MAX_DRAIN_WAITS]
        rest = waits[_MAX_DRAIN_WAITS:]
        while rest:
            extra = nc.sync.drain()
            extra.ins.sync_info = mybir.SyncInfo(
                on_update=[], on_wait=rest[:_MAX_DRAIN_WAITS]
            )
            rest = rest[_MAX_DRAIN_WAITS:]
    nc.all_engine_barrier()
    assert self.sems is not None
    popped = nc._tile_sem_poison_stack.pop()
    assert popped is self._sem_poison
    nc.clear_and_free_semaphores(list(self.sems.allocated().values()))
    nc.all_engine_barrier()


tile.TileContext._drain_and_barrier = _split_drain_and_barrier


def _hoist_excess_waits(nc, max_waits=1):
    """Same walrus limitation, general case: any instruction may carry at most
    `max_waits` sync-wait commands. Hoist overflow waits onto same-engine NOPs
    inserted immediately before the instruction (engine streams are in-order,
    so the NOP blocking on the extra sems is equivalent)."""
    n_split = 0
    for fn in nc.m.functions:
        for blk in fn.blocks:
            insts = list(blk.instructions)
            new_insts = []
            changed = False
            for inst in insts:
                si = inst.sync_info
                waits = list(si.on_wait) if si is not None and si.on_wait else []
                if len(waits) > max_waits:
                    overflow = waits[: len(waits) - max_waits]
                    si.on_wait = waits[len(overflow) :]
                    for k in range(0, len(overflow), max_waits):
                        chunk = overflow[k : k + max_waits]
                        nop = mybir.InstNoOp(
                            name=nc.get_next_instruction_name(),
                            engine=inst.engine,
                            sync_info=mybir.SyncInfo(on_wait=chunk, on_update=[]),
                            bass_nofuse=True,
                        )
                        new_insts.append(nop)
                    changed = True
                    n_split += 1
                new_insts.append(inst)
            if changed:
                blk.instructions = new_insts
    return n_split
# ---------------------------------------------------------------------------

f32 = mybir.dt.float32
bf16 = mybir.dt.bfloat16

B, S, H, I = 4, 2048, 4096, 11008
NCORES = 8
M = (B * S) // NCORES  # 1024 tokens per core
P = 128
KT = H // P  # 32 contraction tiles for gate/up
IT = I // P  # 86 i tiles
TOKB = 512  # token half (stage A psum free size, stage B pass size)
NMB = M // TOKB  # 2
ICH = P  # one i-tile per stage-A weight chunk
NICH = I // ICH  # 86
HOB = 512  # h-out block (stage B psum free size)
NHO = H // HOB  # 8
TPB = TOKB // P  # 4 token tiles per pass
BCH = 8  # i-tiles per stage-B w_down chunk (86 = 10*8 + 6)


def _stage_b_chunks():
    chunks = []
    c0 = 0
    while c0 < IT:
        clen = min(BCH, IT - c0)
        chunks.append((c0, clen))
        c0 += clen
    return chunks


def _build_mlp(tc, xT, wgu, wdp, y):
    nc = tc.nc
    chunks = _stage_b_chunks()

    with tc.tile_pool(name="dram", bufs=1, space="DRAM") as dram_pool:
        # DRAM spill for the second token-half of h only.
        h_dram = dram_pool.tile([P, IT, TOKB], bf16)

        # Long-lived stage-B pools open first so stage-B prefetches can
        # overlap stage A (no address reuse against stage-A pools).
        with tc.tile_pool(name="hslab", bufs=1) as slab_pool, tc.tile_pool(
            name="wdc", bufs=2
        ) as wdc_pool, tc.tile_pool(name="ysb", bufs=2) as y_pool:
            # h slab for the current pass's 512 tokens: [i-part, it, tok]
            hs = slab_pool.tile([P, IT, TOKB], bf16)

            pref_wd = []

            with tc.tile_pool(name="xTp", bufs=1) as xT_pool, tc.tile_pool(
                name="wAb", bufs=2
            ) as wA_bf, tc.tile_pool(name="sgp", bufs=1) as sg_pool, tc.tile_pool(
                name="hAp", bufs=2
            ) as hA_pool, tc.tile_pool(
                name="psA", bufs=3, space="PSUM"
            ) as psumA:
                # xT load [P, KT, M] by token half, interleaved with the
                # first weight chunks on the SP ring so the first matmul can
                # start as soon as half 0 + chunk 0 land.
                # Startup critical path: ONLY the transfers the first 32
                # matmuls need are queued at t=0 (SDMA round-robins between
                # queues, so every extra queued byte delays the critical
                # ones).  sync: xT kt-halves; scalar: wgu0 kt-halves.
                # Everything else (xT half 1, wgu1+, wd prefetches) is
                # emitted later in the ic loop.
                xT_sb = xT_pool.tile([P, NMB, KT, TOKB], bf16)
                kh = KT // 2
                nc.sync.dma_start(xT_sb[:, 0, 0:kh, :], xT[0, :, 0:kh, :])
                nc.sync.dma_start(xT_sb[:, 0, kh:KT, :], xT[0, :, kh:KT, :])
                wbs = {}
                wb0 = wA_bf.tile([P, KT, 2, ICH], bf16, tag="wgu", name="wb")
                nc.scalar.dma_start(wb0[:, 0:kh], wgu[0, :, 0:kh])
                nc.scalar.dma_start(wb0[:, kh:KT], wgu[0, :, kh:KT])
                wbs[0] = wb0
                wb1 = wA_bf.tile([P, KT, 2, ICH], bf16, tag="wgu", name="wb")
                nc.scalar.dma_start(wb1[:], wgu[1])
                wbs[1] = wb1
                # xT half 1 is third in the sync-ring FIFO: it cannot delay
                # the critical xT0 halves, and it must be emitted before its
                # first reader (ic 0, mb 1).
                nc.sync.dma_start(xT_sb[:, 1], xT[1])

                # -- Stage A: h = silu(x@wg) * (x@wu) --
                for ic in range(NICH):
                    if ic == 2:
                        # w_down chunks for pass 0 / ho 0 have no producers;
                        # prefetch on the now-idle SWDGE path.
                        for j in range(2):
                            c0, clen = chunks[j]
                            wdb = wdc_pool.tile(
                                [P, BCH, HOB], bf16, tag="wdc", name="wdbp"
                            )
                            nc.gpsimd.dma_start(
                                wdb[:, :clen, :], wdp[0, :, c0 : c0 + clen, :]
                            )
                            pref_wd.append(wdb)
                    if ic in wbs:
                        wb = wbs[ic]
                    else:
                        wb = wA_bf.tile([P, KT, 2, ICH], bf16, tag="wgu", name="wb")
                        nc.sync.dma_start(wb[:], wgu[ic])
                    for mb in range(NMB):
                        pg = psumA.tile([P, TOKB], f32, tag="pg")
                        pu = psumA.tile([P, TOKB], f32, tag="pu")
                        if ic == 0 and mb == 0:
                            kt_phases = [range(0, kh), range(kh, KT)]
                        else:
                            kt_phases = [range(KT)]
                        for phase in kt_phases:
                            for g, ps in ((0, pg), (1, pu)):
                                for kt in phase:
                                    nc.tensor.matmul(
                                        ps[:],
                                        wb[:, kt, g, :],
                                        xT_sb[:, mb, kt, :],
                                        start=(kt == 0),
                                        stop=(kt == KT - 1),
                                    )
                        sg = sg_pool.tile([P, TOKB], bf16)
                        nc.scalar.activation(
                            sg[:], pg[:], mybir.ActivationFunctionType.Silu
                        )
                        if mb == 0:
                            # first token half: straight into the SBUF slab
                            nc.vector.tensor_mul(
                                out=hs[:, ic, :], in0=sg[:], in1=pu[:]
                            )
                        else:
                            ht = hA_pool.tile([P, TOKB], bf16)
                            nc.vector.tensor_mul(out=ht[:], in0=sg[:], in1=pu[:])
                            nc.scalar.dma_start(h_dram[:, ic, :], ht[:])

            # -- Stage B: y = h @ w_down, two 512-token passes --
            with tc.tile_pool(name="p1c", bufs=1) as p1c_pool, tc.tile_pool(
                name="psB", bufs=2, space="PSUM"
            ) as psumB:
                p1c = None
                for mh in range(NMB):
                    if mh > 0:
                        # reload the slab with the spilled token half; subtile
                        # WAR deps let chunk reloads chase pass-0's last reads
                        for c0, clen in chunks:
                            nc.sync.dma_start(
                                hs[:, c0 : c0 + clen, :],
                                h_dram[:, c0 : c0 + clen, :],
                            )
                    for ho in range(NHO):
                        if mh == 0 and ho == 1:
                            # side-load pass-1's first h chunk on SWDGE: no
                            # WAR against the slab, so it can't be late
                            c0p, clp = chunks[0]
                            p1c = p1c_pool.tile([P, BCH, TOKB], bf16)
                            nc.gpsimd.dma_start(
                                p1c[:, :clp, :], h_dram[:, c0p : c0p + clp, :]
                            )
                        hosl = bass.ds(ho * HOB, HOB)
                        ypsums = [
                            psumB.tile([P, HOB], f32, tag=f"y{tp}", name=f"yps{tp}")
                            for tp in range(TPB)
                        ]
                        for j, (c0, clen) in enumerate(chunks):
                            if mh == 0 and ho == 0 and j < len(pref_wd):
                                wdb = pref_wd[j]
                            else:
                                wdb = wdc_pool.tile(
                                    [P, BCH, HOB], bf16, tag="wdc", name="wdb"
                                )
                                nc.scalar.dma_start(
                                    wdb[:, :clen, :],
                                    wdp[ho, :, c0 : c0 + clen, :],
                                )
                            hsrc = p1c if (mh == 1 and ho == 0 and j == 0) else None
                            for tp in range(TPB):
                                for il in range(clen):
                                    nc.tensor.matmul(
                                        ypsums[tp][:],
                                        hsrc[:, il, tp * P : (tp + 1) * P]
                                        if hsrc is not None
                                        else hs[:, c0 + il, tp * P : (tp + 1) * P],
                                        wdb[:, il, :],
                                        start=(j == 0 and il == 0),
                                        stop=(
                                            j == len(chunks) - 1
                                            and il == clen - 1
                                        ),
                                    )
                        for tp in range(TPB):
                            yt = y_pool.tile([P, HOB], f32)
                            if tp % 2 == 0:
                                nc.scalar.copy(yt[:], ypsums[tp][:])
                            else:
                                nc.vector.tensor_copy(yt[:], ypsums[tp][:])
                            ysl = y[
                                mh * TOKB + tp * P : mh * TOKB + (tp + 1) * P, hosl
                            ]
                            if mh == NMB - 1 and ho == NHO - 1:
                                yeng = nc.sync if tp % 2 == 0 else nc.scalar
                                yeng.dma_start(ysl, yt[:])
                            else:
                                nc.gpsimd.dma_start(ysl, yt[:])


_NC_CACHE = None


def _build():
    global _NC_CACHE
    if _NC_CACHE is not None:
        return _NC_CACHE
    nc = bass.Bass(num_swdge_queues=4)
    xT = nc.dram_tensor("xT", [NMB, P, KT, TOKB], bf16, kind="ExternalInput")
    wgu = nc.dram_tensor("wgu", [NICH, P, KT, 2, ICH], bf16, kind="ExternalInput")
    wdp = nc.dram_tensor("wdp", [NHO, P, IT, HOB], bf16, kind="ExternalInput")
    y = nc.dram_tensor("y", [M, H], f32, kind="ExternalOutput")
    with tile.TileContext(nc) as tc:
        _build_mlp(tc, xT, wgu, wdp, y)
    _hoist_excess_waits(nc)
    _NC_CACHE = nc
    return nc


LAST_RESULTS = None


def kernel(x, w_gate, w_up, w_down):
    global LAST_RESULTS
    bf = ml_dtypes.bfloat16
    x = np.asarray(x, dtype=np.float32).reshape(B * S, H)
    w_gate = np.asarray(w_gate, dtype=np.float32)
    w_up = np.asarray(w_up, dtype=np.float32)
    w_down = np.asarray(w_down, dtype=np.float32)

    # Packed layouts: every device DMA reads fully-contiguous per-partition
    # byte ranges.
    # wgu[ic, p, kt, g, i] = {wg,wu}[kt*P + p, ic*ICH + i]
    wgr = w_gate.reshape(KT, P, NICH, ICH).transpose(2, 1, 0, 3)
    wur = w_up.reshape(KT, P, NICH, ICH).transpose(2, 1, 0, 3)
    wgu = np.ascontiguousarray(
        np.stack([wgr, wur], axis=3).astype(bf)
    )  # [NICH, P, KT, 2, ICH]
    # wdp[ho, p, it, hb] = wd[it*P + p, ho*HOB + hb]
    wdp = np.ascontiguousarray(
        w_down.reshape(IT, P, NHO, HOB).transpose(2, 1, 0, 3).astype(bf)
    )
    # xT[p, kt, m] = x[m, kt*P + p], per core slice
    xTs = [
        np.ascontiguousarray(
            x[c * M : (c + 1) * M]
            .reshape(NMB, TOKB, KT, P)
            .transpose(0, 3, 2, 1)
            .astype(bf)
        )
        for c in range(NCORES)
    ]

    nc = _build()
    in_maps = [
        {"xT": xTs[c], "wgu": wgu, "wdp": wdp}
        for c in range(NCORES)
    ]
    trace = os.environ.get("KERNEL_TRACE") == "1"
    res = run_bass_kernel_spmd(
        nc, in_maps, core_ids=list(range(NCORES)), trace=trace
    )
    LAST_RESULTS = res
    if res.exec_time_ns is not None:
        print(f"HW exec time: {res.exec_time_ns} ns")
    y = np.concatenate([r["y"] for r in res.results], axis=0)
    return y.reshape(B, S, H)

